# revision 20
# baseline (speedup 1.0000x reference)
"""Ensemble-SRN MoE routing kernel for 8 TRN2 NeuronCores.

Strategy: expert-parallel sharding. The 8 experts are axis-aligned octants of
[-1,1]^3 (GRID=(2,2,2)); core e receives exactly the points routed to expert e
(the all-to-all dispatch happens on the host as part of sharding), runs a dense
single-expert 3->64->64->1 ReLU MLP over its (padded) shard, and the host
inverse-permutes the outputs.

Device dataflow: the whole shard's x (bf16) and y (f32) stay resident in
SBUF, so HBM traffic is 4 input DMAs (prefetched on the Pool/SWDGE queue)
plus a few chunked output DMAs per pass -- each DMACopy costs a flat ~625ns
on the single HWDGE queue, which was the original bottleneck. In bench mode
the whole pass is ping-pong double-buffered across a 2x-unrolled hardware
loop so consecutive passes overlap.

Per "round" of 4096 points (4 pairs x 2 slots x 512), weights block-diag
doubled over the two slots so every 512-column PE stream covers 1024 points:
  L1 (K=6 ->M=128): row-quadrant matmuls at tile_position (32p, 0), in TWO
                    WAVES through a single [128,1024] psum tile (2 banks):
                    pairs 0,1 write, DVE evacuates, pairs 2,3 overwrite,
                    ACT evacuates -- halving L1's psum footprint to free
                    banks for a dedicated y pool
  L2 (K=64 ->M=64): 2 concurrent 64x64-quadrant matmuls per pair; even pairs
                    use array tiles (0,0)/(64,64), odd pairs (0,64)/(64,0)
                    (odd pairs' slot halves land swapped in psum; the w3
                    variants below restore slot order), two [128,1024] psum
                    tiles (pool bufs=2, 4 banks)
  L3 (K=128->M=4):  col-quadrant matmuls into a dedicated [128,512] psum
                    pool (bufs=2): EVEN rounds open an accumulation group
                    writing y into partitions 32p+{0,1} (w3 cols 0,1), ODD
                    rounds close it into 32p+{2,3} (cols 2,3) -- two rounds
                    share one bank and one bias-add, halving y-evac cost.
                    Emitted one round late so engine orders interleave.
  PSUM evac split (the true pacing resource): DVE takes pair 0,1 columns of
  h1/h2, ACT pairs 2,3 plus the per-2-round y bias-add (~2.7 us each).
A final partial round of `tp` pairs (1024 points each) handles the remainder
beyond 4096*nrf, so the shard capacity is nrf*4096 + tp*1024.
"""

import ml_dtypes
import numpy as np

import concourse.bass as bass
import concourse.tile as tile
from concourse import bacc, mybir
from concourse.bass_utils import run_bass_kernel_spmd

F32 = mybir.dt.float32
BF16 = mybir.dt.bfloat16

N_CORES = 8
GRID = (2, 2, 2)
H = 64
F = 512                  # points per tile (one PSUM-bank free dim, fp32)
PAIR = 2 * F             # points per pair (two slots, block-diag)
ROUND = 4 * PAIR         # 4096 points per full round

_PROGRAM_CACHE = {}
LAST_RESULTS = None   # BassKernelResults of the last run (for test harness)
LAST_IN_MAPS = None   # per-core input dicts of the last run (for test harness)
LAST_NC = None        # compiled program of the last run (for test harness)
LAST_SHAPE_KEY = None # (nrf, tp) of the last run (for test harness)


def _build_program(shape_key, loop_n=None, stage="full"):
    """Build the SPMD program for shape_key=(nrf, tp): nrf full rounds plus a
    tail partial round of tp pairs. loop_n (bench only): repeat the whole body
    loop_n times in a hardware For_i so device time can be measured through
    the noisy axon dispatch path by differencing two loop counts."""
    nrf, tp = shape_key
    n_sub = nrf + (1 if tp else 0)
    W = n_sub * F
    n_grp = (n_sub + 1) // 2
    Wy = n_grp * F
    nc = bacc.Bacc(
        "TRN2",
        target_bir_lowering=False,
        debug=False,
        num_devices=N_CORES,
    )
    xT = nc.dram_tensor("xT", [4, 6, W], BF16, kind="ExternalInput")
    wp = nc.dram_tensor("wp", [128, 272], BF16, kind="ExternalInput")
    bp = nc.dram_tensor("bp", [128, 3], F32, kind="ExternalInput")
    yO = nc.dram_tensor("y", [4, 4, Wy], F32, kind="ExternalOutput")

    RELU = mybir.ActivationFunctionType.Relu
    IDENT = mybir.ActivationFunctionType.Identity
    ADD = mybir.AluOpType.add
    MAX = mybir.AluOpType.max

    def pairs_in(r):
        return 4 if r < nrf else tp

    with tile.TileContext(nc) as tc:
        with (
            tc.tile_pool(name="const", bufs=1) as const,
            tc.tile_pool(name="xin", bufs=2) as xin,
            tc.tile_pool(name="yout", bufs=2) as yout,
            tc.tile_pool(name="h1p", bufs=3) as h1pool,
            tc.tile_pool(name="h2p", bufs=3) as h2pool,
            tc.tile_pool(name="psH1", bufs=1, space="PSUM") as psH1,
            tc.tile_pool(name="psH2", bufs=2, space="PSUM") as psH2,
            tc.tile_pool(name="psY", bufs=2, space="PSUM") as psY,
        ):
            w_sb = const.tile([128, 272], BF16)
            nc.sync.dma_start(w_sb[:], wp.ap())
            b_sb = const.tile([128, 3], F32)
            nc.sync.dma_start(b_sb[:], bp.ap())

            w1s = w_sb[:, 0:128]
            w2s = w_sb[:, 128:256]
            # w3 variants: [even/odd round][even/odd pair] -> [128,4]
            w3v = {
                (True, 0): w_sb[:, 256:260],
                (True, 1): w_sb[:, 260:264],
                (False, 0): w_sb[:, 264:268],
                (False, 1): w_sb[:, 268:272],
            }
            b1s = b_sb[:, 0:1]
            b2s = b_sb[:, 1:2]
            b3s = b_sb[:, 2:3]

            # Load the Relu/Identity activation table once, outside the loop,
            # so the per-pass body doesn't pay the ~1.3us table load.
            warm_a = const.tile([128, 1], F32)
            warm_b = const.tile([128, 1], F32)
            nc.vector.memset(warm_a[:], 0.0)
            nc.scalar.activation(
                warm_b[:], warm_a[:], mybir.ActivationFunctionType.Relu
            )
            nc.scalar.activation(
                warm_a[:], warm_b[:], mybir.ActivationFunctionType.Identity
            )

            import contextlib
            if loop_n:
                assert loop_n % 2 == 0 and loop_n >= 2, loop_n
                loop_cm = tc.For_i(
                    0, loop_n // 2, 1,
                    hint_engines=(
                        mybir.EngineType.PE,
                        mybir.EngineType.DVE,
                        mybir.EngineType.Activation,
                        mybir.EngineType.SP,
                        mybir.EngineType.Pool,
                    ),
                )
            else:
                loop_cm = contextlib.nullcontext()

            def emit_pass():
                # Ping-pong whole-shard buffers: pass i+1's x prefetch (on the
                # idle Pool/SWDGE queue, so it doesn't serialize behind the y
                # stores on SP) overlaps pass i's compute.
                x_sb = xin.tile([128, W], BF16, name="x_sb")
                y_sb = yout.tile([128, Wy], F32, name="y_sb")
                for p in range(4):
                    nc.gpsimd.dma_start(
                        x_sb[32 * p : 32 * p + 6, :], xT.ap()[p]
                    )

                grp = {"phy": None, "np_next": 0, "yc0": 0}

                def emit_l3_y(prev):
                    h2r_p, q, np_q = prev
                    even = (q % 2) == 0
                    if even:
                        grp["phy"] = psY.tile(
                            [128, 512], F32, tag="phy", name="phy"
                        )
                        grp["np_next"] = pairs_in(q + 1) if q + 1 < n_sub else 0
                    phy = grp["phy"]
                    for p in range(np_q):
                        nc.tensor.matmul(
                            phy[32 * p : 32 * p + 4, 0:512],
                            w3v[(even, p % 2)],
                            h2r_p[:, F * p : F * p + F],
                            start=even,
                            stop=(p >= grp["np_next"]) if even else True,
                            tile_position=(0, 32 * p),
                        )
                    if (not even) or q == n_sub - 1:
                        g = q // 2
                        nc.scalar.activation(
                            y_sb[:, F * g : F * g + F], phy[:, 0:512],
                            IDENT, bias=b3s,
                        )
                        # chunked y store: flush every 4 groups
                        if g == n_grp - 1 or (g % 4) == 3:
                            c1 = F * g + F
                            for p in range(4):
                                nc.sync.dma_start(
                                    yO.ap()[p, :, grp["yc0"] : c1],
                                    y_sb[32 * p : 32 * p + 4, grp["yc0"] : c1],
                                )
                            grp["yc0"] = c1

                prev = None
                for r in range(n_sub):
                    np_r = pairs_in(r)
                    cols = slice(F * r, F * r + F)
                    w1h = min(2, np_r)

                    # ---- L1 wave 1: pairs 0..w1h into the 2-bank ph1 ----
                    ph1 = psH1.tile([128, 1024], F32, tag="ph1")
                    for p in range(w1h):
                        nc.tensor.matmul(
                            ph1[:, F * p : F * p + F],
                            w1s[32 * p : 32 * p + 6, :],
                            x_sb[32 * p : 32 * p + 6, cols],
                            start=True,
                            stop=True,
                            tile_position=(32 * p, 0),
                        )

                    # ---- delayed L3/y of the previous round ----
                    if prev is not None:
                        emit_l3_y(prev)

                    # DVE evacuates wave 1 while the PE refills ph1
                    h1r = h1pool.tile([128, 2048], BF16)
                    d_end = w1h * F
                    nc.vector.tensor_scalar(
                        h1r[:, 0:d_end], ph1[:, 0:d_end], b1s, 0.0, ADD, MAX
                    )

                    # ---- L1 wave 2: pairs 2..np_r overwrite ph1 ----
                    for p in range(2, np_r):
                        nc.tensor.matmul(
                            ph1[:, F * (p - 2) : F * (p - 2) + F],
                            w1s[32 * p : 32 * p + 6, :],
                            x_sb[32 * p : 32 * p + 6, cols],
                            start=True,
                            stop=True,
                            tile_position=(32 * p, 0),
                        )
                    if np_r > 2:
                        nc.scalar.activation(
                            h1r[:, 1024 : F * np_r],
                            ph1[:, 0 : F * (np_r - 2)],
                            RELU, bias=b1s,
                        )

                    # ---- L2: 2 concurrent 64x64-quadrant matmuls per pair
                    # (even pairs use tiles (0,0)/(64,64); odd pairs use
                    # (0,64)/(64,0), landing their slots swapped in psum --
                    # the w3 variants put y back in slot order) ----
                    h2r = h2pool.tile([128, 2048], BF16)
                    ph2a = psH2.tile([128, 1024], F32, tag="ph2")
                    ph2b = (
                        psH2.tile([128, 1024], F32, tag="ph2", name="ph2b")
                        if np_r > 2 else None
                    )
                    for p in range(np_r):
                        dst = ph2a if p < 2 else ph2b
                        c = slice(F * (p % 2), F * (p % 2) + F)
                        for s in range(2):
                            if p % 2 == 0:
                                out_half = dst[64 * s : 64 * s + 64, c]
                                wq = w2s[64 * s : 64 * s + 64,
                                         64 * s : 64 * s + 64]
                                tpos = (64 * s, 64 * s)
                            else:
                                out_half = dst[64 - 64 * s : 128 - 64 * s, c]
                                wq = w2s[64 * s : 64 * s + 64,
                                         64 - 64 * s : 128 - 64 * s]
                                tpos = (64 * s, 64 - 64 * s)
                            nc.tensor.matmul(
                                out_half,
                                wq,
                                h1r[64 * s : 64 * s + 64, F * p : F * p + F],
                                start=True,
                                stop=True,
                                tile_position=tpos,
                            )
                    nc.vector.tensor_scalar(
                        h2r[:, 0:d_end], ph2a[:, 0:d_end], b2s, 0.0, ADD, MAX
                    )
                    if np_r > 2:
                        nc.scalar.activation(
                            h2r[:, 1024 : F * np_r],
                            ph2b[:, 0 : F * (np_r - 2)],
                            RELU, bias=b2s,
                        )

                    prev = (h2r, r, np_r)

                emit_l3_y(prev)

            with loop_cm:
                emit_pass()
                if loop_n:
                    emit_pass()

    nc.compile()
    return nc


def kernel(x, extents_min, extents_max, W1, b1, W2, b2, W3, b3):
    global LAST_RESULTS, LAST_IN_MAPS, LAST_NC, LAST_SHAPE_KEY
    x = np.ascontiguousarray(np.asarray(x, dtype=np.float32))
    extents_min = np.asarray(extents_min, dtype=np.float32)
    extents_max = np.asarray(extents_max, dtype=np.float32)
    W1 = np.asarray(W1, dtype=np.float32)
    b1 = np.asarray(b1, dtype=np.float32)
    W2 = np.asarray(W2, dtype=np.float32)
    b2 = np.asarray(b2, dtype=np.float32)
    W3 = np.asarray(W3, dtype=np.float32)
    b3 = np.asarray(b3, dtype=np.float32)

    n_pts = x.shape[0]
    E = W1.shape[0]
    assert E == N_CORES

    # --- routing (identical fp32 math to the reference) ---
    gvec = np.asarray(GRID, dtype=np.float32)
    u = np.clip((x + np.float32(1.0)) * np.float32(0.5), 0.0, 0.99)
    gi = (u * gvec).astype(np.int32)
    idx = gi[:, 0] + gi[:, 1] * GRID[0] + gi[:, 2] * (GRID[0] * GRID[1])

    order = np.argsort(idx, kind="stable")
    counts = np.bincount(idx, minlength=E)
    starts = np.concatenate([[0], np.cumsum(counts)[:-1]])
    x_sorted = x[order]

    total_pairs = max(1, int(np.ceil(counts.max() / PAIR)))
    nrf, tp = total_pairs // 4, total_pairs % 4
    if nrf == 0:
        nrf, tp = 1, 0
    cap = nrf * ROUND + tp * PAIR
    n_sub = nrf + (1 if tp else 0)
    W = n_sub * F
    n_grp = (n_sub + 1) // 2

    # --- fold the expert-local normalization into layer-1 weights ---
    # xn = s*x + t, s = 2/(emax-emin), t = -2*emin/(emax-emin) - 1
    span = extents_max - extents_min          # [E, 3]
    s = 2.0 / span
    tvec = -2.0 * extents_min / span - 1.0
    # h1_pre = x @ W1e' + b1e',  W1e' = diag(s) @ W1e, b1e' = b1e + t @ W1e
    W1p = W1 * s[:, :, None]                  # [E, 3, H]
    b1p = b1 + np.einsum("ec,ech->eh", tvec, W1)

    in_maps = []
    for e in range(E):
        xe = np.zeros((cap, 3), dtype=np.float32)
        xe[: counts[e]] = x_sorted[starts[e] : starts[e] + counts[e]]
        # xt[p, 3s+c, r*512+n] = xe[r*4096 + (2p+s)*512 + n, c]
        xt = np.zeros((4, 6, W), dtype=ml_dtypes.bfloat16)
        main = (
            xe[: nrf * ROUND]
            .reshape(nrf, 4, 2, F, 3)         # r, p, s, n, c
            .transpose(1, 2, 4, 0, 3)         # p, s, c, r, n
            .reshape(4, 6, nrf * F)
        )
        xt[:, :, : nrf * F] = main.astype(ml_dtypes.bfloat16)
        if tp:
            tail = (
                xe[nrf * ROUND :]
                .reshape(tp, 2, F, 3)         # t, s, n, c
                .transpose(0, 1, 3, 2)        # t, s, c, n
                .reshape(tp, 6, F)
            )
            xt[:tp, :, nrf * F :] = tail.astype(ml_dtypes.bfloat16)

        # w1: 4 row strips (one per pair), each the [6,128] block-diag of W1'
        w1e = W1p[e].astype(ml_dtypes.bfloat16)
        w1_full = np.zeros((128, 128), dtype=ml_dtypes.bfloat16)
        for p in range(4):
            w1_full[32 * p : 32 * p + 3, 0:64] = w1e
            w1_full[32 * p + 3 : 32 * p + 6, 64:128] = w1e
        # w2: all four [64,64] quadrants hold W2 (L2 runs as 64x64-tile
        # matmuls reading whichever quadrant matches its tile position)
        w2_full = np.tile(W2[e].astype(ml_dtypes.bfloat16), (2, 2))
        # w3 variants [even/odd round][even/odd pair]: even rounds write y
        # into cols {0,1}, odd rounds into cols {2,3}; odd pairs' slot
        # halves are swapped in psum after the quadrant L2, so their
        # partition-half -> col mapping is flipped.
        w3c = W3[e, :, 0].astype(ml_dtypes.bfloat16)
        w3_parts = []
        for even in (True, False):
            base = 0 if even else 2
            for pp in (0, 1):
                m = np.zeros((128, 4), dtype=ml_dtypes.bfloat16)
                if pp == 0:
                    m[0:64, base + 0] = w3c
                    m[64:128, base + 1] = w3c
                else:
                    m[64:128, base + 0] = w3c
                    m[0:64, base + 1] = w3c
                w3_parts.append(m)
        wpack = np.ascontiguousarray(
            np.concatenate([w1_full, w2_full] + w3_parts, axis=1)
        )
        b1_full = np.tile(b1p[e], 2).astype(np.float32)
        b2_full = np.tile(b2[e], 2).astype(np.float32)
        b3_full = np.full(128, b3[e, 0], dtype=np.float32)
        bpack = np.ascontiguousarray(
            np.stack([b1_full, b2_full, b3_full], axis=1)
        )
        in_maps.append({"xT": np.ascontiguousarray(xt), "wp": wpack,
                        "bp": bpack})

    shape_key = (nrf, tp)
    if shape_key not in _PROGRAM_CACHE:
        _PROGRAM_CACHE[shape_key] = _build_program(shape_key)
    nc = _PROGRAM_CACHE[shape_key]

    res = run_bass_kernel_spmd(nc, in_maps, core_ids=list(range(N_CORES)))
    LAST_RESULTS = res
    LAST_IN_MAPS = in_maps
    LAST_NC = nc
    LAST_SHAPE_KEY = shape_key

    # --- unshard: y[p, 2*(r%2)+s, (r//2)*512+n] -> r*4096+(2p+s)*512+n ---
    y_sorted = np.empty(n_pts, dtype=np.float32)
    for e in range(E):
        yd = res.results[e]["y"]              # [4, 4, Wy]
        flat = (
            yd.reshape(4, 2, 2, n_grp, F)     # p, odd, s, g, n
            .transpose(3, 1, 0, 2, 4)         # g, odd, p, s, n
            .reshape(n_grp * 2 * ROUND)
        )
        y_sorted[starts[e] : starts[e] + counts[e]] = flat[: counts[e]]

    y_full = np.empty(n_pts, dtype=np.float32)
    y_full[order] = y_sorted
    return y_full[:, None]


# revision 21
# speedup vs baseline: 1.1268x; 1.1268x over previous
"""Ensemble-SRN MoE routing kernel for 8 TRN2 NeuronCores.

Strategy: expert-parallel sharding. The 8 experts are axis-aligned octants of
[-1,1]^3 (GRID=(2,2,2)); core e receives exactly the points routed to expert e
(the all-to-all dispatch happens on the host as part of sharding), runs a dense
single-expert 3->64->64->1 ReLU MLP over its (padded) shard, and the host
inverse-permutes the outputs.

Device dataflow (v3): the whole shard's x (bf16) and y (f32) stay resident in
SBUF, so HBM traffic is 4 input DMAs + a handful of chunked output DMAs per
pass (the v1 bottleneck was ~625ns per DMACopy on the single HWDGE queue).

Per "round" of 4096 points (4 pairs x 2 slots x 512), weights block-diag
doubled over the two slots so every 512-column PE stream covers 1024 points:
  L1 (K=6 ->M=128): 4 row-quadrant matmuls, pair p at tile_position (32p, 0),
                    into two [128,1024] psum tiles (pool bufs=2)
  L2 (K=128->M=128): 4 full-array matmuls into per-pair [128,512] psum tiles
                    (pool bufs=3) so each pair's evac/L3 proceeds alone
  L3 (K=128->M=4):  4 col-quadrant matmuls; EVEN rounds open a psum
                    accumulation group writing y into partitions 32p+{0,1}
                    (w3 cols 0,1), ODD rounds close it into 32p+{2,3}
                    (w3 cols 2,3) -- two rounds of y share one bank and one
                    bias-add evac, halving the y-evac cost on ACT.
  PSUM evac split: DVE takes pairs 0,1 (h1a + h2 p0,p1), ACT pairs 2,3 plus
  the per-2-round y bias-add: ~2.5us vs ~2.4us per round against PE's 2.56us.
A final partial round of `tp` pairs (1024 points each) handles the remainder
beyond 4096*nrf, so the shard capacity is nrf*4096 + tp*1024.
"""

import ml_dtypes
import numpy as np

import concourse.bass as bass
import concourse.tile as tile
from concourse import bacc, mybir
from concourse.bass_utils import run_bass_kernel_spmd

F32 = mybir.dt.float32
BF16 = mybir.dt.bfloat16

N_CORES = 8
GRID = (2, 2, 2)
H = 64
F = 512                  # points per tile (one PSUM-bank free dim, fp32)
PAIR = 2 * F             # points per pair (two slots, block-diag)
ROUND = 4 * PAIR         # 4096 points per full round

_PROGRAM_CACHE = {}
LAST_RESULTS = None   # BassKernelResults of the last run (for test harness)
LAST_IN_MAPS = None   # per-core input dicts of the last run (for test harness)
LAST_NC = None        # compiled program of the last run (for test harness)
LAST_SHAPE_KEY = None # (nrf, tp) of the last run (for test harness)


def _build_program(shape_key, loop_n=None, stage="full"):
    """Build the SPMD program for shape_key=(nrf, tp): nrf full rounds plus a
    tail partial round of tp pairs. loop_n (bench only): repeat the whole body
    loop_n times in a hardware For_i so device time can be measured through
    the noisy axon dispatch path by differencing two loop counts."""
    nrf, tp = shape_key
    n_sub = nrf + (1 if tp else 0)
    W = n_sub * F
    nc = bacc.Bacc(
        "TRN2",
        target_bir_lowering=False,
        debug=False,
        num_devices=N_CORES,
    )
    xT = nc.dram_tensor("xT", [4, 6, W], BF16, kind="ExternalInput")
    wp = nc.dram_tensor("wp", [128, 264], BF16, kind="ExternalInput")
    bp = nc.dram_tensor("bp", [128, 3], F32, kind="ExternalInput")
    yO = nc.dram_tensor("y", [4, 2, W], F32, kind="ExternalOutput")

    RELU = mybir.ActivationFunctionType.Relu
    IDENT = mybir.ActivationFunctionType.Identity
    ADD = mybir.AluOpType.add
    MAX = mybir.AluOpType.max

    def pairs_in(r):
        return 4 if r < nrf else tp

    with tile.TileContext(nc) as tc:
        with (
            tc.tile_pool(name="const", bufs=1) as const,
            tc.tile_pool(name="xin", bufs=2) as xin,
            tc.tile_pool(name="yout", bufs=2) as yout,
            tc.tile_pool(name="h1p", bufs=3) as h1pool,
            tc.tile_pool(name="h2p", bufs=3) as h2pool,
            tc.tile_pool(name="psH1", bufs=2, space="PSUM") as psH1,
            tc.tile_pool(name="psH2", bufs=2, space="PSUM") as psH2,
        ):
            w_sb = const.tile([128, 264], BF16)
            nc.sync.dma_start(w_sb[:], wp.ap())
            b_sb = const.tile([128, 3], F32)
            nc.sync.dma_start(b_sb[:], bp.ap())

            w1s = w_sb[:, 0:128]
            w2s = w_sb[:, 128:256]
            w3e = w_sb[:, 256:260]
            w3o = w_sb[:, 260:264]
            b1s = b_sb[:, 0:1]
            b2s = b_sb[:, 1:2]
            b3s = b_sb[:, 2:3]

            # Load the Relu/Identity activation table once, outside the loop,
            # so the per-pass body doesn't pay the ~1.3us table load.
            warm_a = const.tile([128, 1], F32)
            warm_b = const.tile([128, 1], F32)
            nc.vector.memset(warm_a[:], 0.0)
            nc.scalar.activation(
                warm_b[:], warm_a[:], mybir.ActivationFunctionType.Relu
            )
            nc.scalar.activation(
                warm_a[:], warm_b[:], mybir.ActivationFunctionType.Identity
            )

            import contextlib
            if loop_n:
                assert loop_n % 2 == 0 and loop_n >= 2, loop_n
                loop_cm = tc.For_i(
                    0, loop_n // 2, 1,
                    hint_engines=(
                        mybir.EngineType.PE,
                        mybir.EngineType.DVE,
                        mybir.EngineType.Activation,
                        mybir.EngineType.SP,
                        mybir.EngineType.Pool,
                    ),
                )
            else:
                loop_cm = contextlib.nullcontext()

            def emit_pass():
                # Ping-pong whole-shard buffers: pass i+1's x prefetch (on the
                # idle Pool/SWDGE queue, so it doesn't serialize behind the y
                # stores on SP) overlaps pass i's compute.
                x_sb = xin.tile([128, W], BF16, name="x_sb")
                y_sb = yout.tile([128, W], F32, name="y_sb")
                for p in range(4):
                    nc.gpsimd.dma_start(
                        x_sb[32 * p : 32 * p + 6, :], xT.ap()[p]
                    )

                # L3(q)+y(q) are emitted one round LATE (during round q+1),
                # so each engine's baked instruction order interleaves two
                # rounds and no op waits on work emitted after it: ACT runs
                # y(q), h1b(q+1), h2b(q+1); PE runs L1(q+1), L3(q), L2(q+1).
                def emit_l3_y(prev, yc0):
                    h2r_p, ph2a_p, q, np_q = prev
                    for p in range(np_q):
                        nc.tensor.matmul(
                            ph2a_p[32 * p : 32 * p + 2, 0:512],
                            w3e[:, 0:2] if p % 2 == 0 else w3o[:, 0:2],
                            h2r_p[:, F * p : F * p + F],
                            start=True,
                            stop=True,
                            tile_position=(0, 32 * p),
                        )
                    nc.scalar.activation(
                        y_sb[:, F * q : F * q + F], ph2a_p[:, 0:512],
                        IDENT, bias=b3s,
                    )
                    # chunked y store: flush every 8 rounds
                    if q == n_sub - 1 or (q % 8) == 7:
                        c1 = F * q + F
                        for p in range(4):
                            nc.sync.dma_start(
                                yO.ap()[p, :, yc0:c1],
                                y_sb[32 * p : 32 * p + 2, yc0:c1],
                            )
                        return c1
                    return yc0

                yc0 = 0
                prev = None
                for r in range(n_sub):
                    np_r = pairs_in(r)
                    cols = slice(F * r, F * r + F)

                    # ---- L1: np_r row-quadrant block-diag matmuls ----
                    ph1a = psH1.tile([128, 1024], F32, tag="ph1")
                    ph1b = (
                        psH1.tile([128, 1024], F32, tag="ph1", name="ph1b")
                        if np_r > 2 else None
                    )
                    for p in range(np_r):
                        dst = ph1a if p < 2 else ph1b
                        nc.tensor.matmul(
                            dst[:, F * (p % 2) : F * (p % 2) + F],
                            w1s[32 * p : 32 * p + 6, :],
                            x_sb[32 * p : 32 * p + 6, cols],
                            start=True,
                            stop=True,
                            tile_position=(32 * p, 0),
                        )

                    # ---- delayed L3/y of the previous round ----
                    if prev is not None:
                        yc0 = emit_l3_y(prev, yc0)

                    # relu + bias evac: DVE takes pairs 0,1; ACT pairs 2,3
                    h1r = h1pool.tile([128, 2048], BF16)
                    d_end = min(2, np_r) * F
                    nc.scalar.activation(
                        h1r[:, 0:d_end], ph1a[:, 0:d_end], RELU, bias=b1s
                    )
                    if np_r > 2:
                        nc.vector.tensor_scalar(
                            h1r[:, 1024 : F * np_r],
                            ph1b[:, 0 : F * (np_r - 2)],
                            b1s, 0.0, ADD, MAX,
                        )

                    # ---- L2: 2 concurrent 64x64-quadrant matmuls per pair
                    # (even pairs use tiles (0,0)/(64,64); odd pairs use
                    # (0,64)/(64,0), landing their slots swapped in psum --
                    # the swapped w3 below puts y back in slot order) ----
                    h2r = h2pool.tile([128, 2048], BF16)
                    ph2a = psH2.tile([128, 1024], F32, tag="ph2")
                    ph2b = (
                        psH2.tile([128, 1024], F32, tag="ph2", name="ph2b")
                        if np_r > 2 else None
                    )
                    for p in range(np_r):
                        dst = ph2a if p < 2 else ph2b
                        c = slice(F * (p % 2), F * (p % 2) + F)
                        for s in range(2):
                            if p % 2 == 0:
                                out_half = dst[64 * s : 64 * s + 64, c]
                                wq = w2s[64 * s : 64 * s + 64,
                                         64 * s : 64 * s + 64]
                                tpos = (64 * s, 64 * s)
                            else:
                                out_half = dst[64 - 64 * s : 128 - 64 * s, c]
                                wq = w2s[64 * s : 64 * s + 64,
                                         64 - 64 * s : 128 - 64 * s]
                                tpos = (64 * s, 64 - 64 * s)
                            nc.tensor.matmul(
                                out_half,
                                wq,
                                h1r[64 * s : 64 * s + 64, F * p : F * p + F],
                                start=True,
                                stop=True,
                                tile_position=tpos,
                            )
                    nc.scalar.activation(
                        h2r[:, 0:d_end], ph2a[:, 0:d_end], RELU, bias=b2s
                    )
                    if np_r > 2:
                        nc.vector.tensor_scalar(
                            h2r[:, 1024 : F * np_r],
                            ph2b[:, 0 : F * (np_r - 2)],
                            b2s, 0.0, ADD, MAX,
                        )

                    # L3(r) writes y into bank C0 of ph2a (fully read by the
                    # h2 evac; its slot isn't rewritten until round r+1's L2)
                    # -- emitted at the top of round r+1.
                    prev = (h2r, ph2a, r, np_r)

                emit_l3_y(prev, yc0)

            with loop_cm:
                emit_pass()
                if loop_n:
                    emit_pass()

    nc.compile()
    return nc


def kernel(x, extents_min, extents_max, W1, b1, W2, b2, W3, b3):
    global LAST_RESULTS, LAST_IN_MAPS, LAST_NC, LAST_SHAPE_KEY
    x = np.ascontiguousarray(np.asarray(x, dtype=np.float32))
    extents_min = np.asarray(extents_min, dtype=np.float32)
    extents_max = np.asarray(extents_max, dtype=np.float32)
    W1 = np.asarray(W1, dtype=np.float32)
    b1 = np.asarray(b1, dtype=np.float32)
    W2 = np.asarray(W2, dtype=np.float32)
    b2 = np.asarray(b2, dtype=np.float32)
    W3 = np.asarray(W3, dtype=np.float32)
    b3 = np.asarray(b3, dtype=np.float32)

    n_pts = x.shape[0]
    E = W1.shape[0]
    assert E == N_CORES

    # --- routing (identical fp32 math to the reference) ---
    gvec = np.asarray(GRID, dtype=np.float32)
    u = np.clip((x + np.float32(1.0)) * np.float32(0.5), 0.0, 0.99)
    gi = (u * gvec).astype(np.int32)
    idx = gi[:, 0] + gi[:, 1] * GRID[0] + gi[:, 2] * (GRID[0] * GRID[1])

    order = np.argsort(idx, kind="stable")
    counts = np.bincount(idx, minlength=E)
    starts = np.concatenate([[0], np.cumsum(counts)[:-1]])
    x_sorted = x[order]

    total_pairs = max(1, int(np.ceil(counts.max() / PAIR)))
    nrf, tp = total_pairs // 4, total_pairs % 4
    if nrf == 0:
        nrf, tp = 1, 0
    cap = nrf * ROUND + tp * PAIR
    n_sub = nrf + (1 if tp else 0)
    W = n_sub * F

    # --- fold the expert-local normalization into layer-1 weights ---
    # xn = s*x + t, s = 2/(emax-emin), t = -2*emin/(emax-emin) - 1
    span = extents_max - extents_min          # [E, 3]
    s = 2.0 / span
    tvec = -2.0 * extents_min / span - 1.0
    # h1_pre = x @ W1e' + b1e',  W1e' = diag(s) @ W1e, b1e' = b1e + t @ W1e
    W1p = W1 * s[:, :, None]                  # [E, 3, H]
    b1p = b1 + np.einsum("ec,ech->eh", tvec, W1)

    in_maps = []
    for e in range(E):
        xe = np.zeros((cap, 3), dtype=np.float32)
        xe[: counts[e]] = x_sorted[starts[e] : starts[e] + counts[e]]
        # xt[p, 3s+c, r*512+n] = xe[r*4096 + (2p+s)*512 + n, c]
        xt = np.zeros((4, 6, W), dtype=ml_dtypes.bfloat16)
        main = (
            xe[: nrf * ROUND]
            .reshape(nrf, 4, 2, F, 3)         # r, p, s, n, c
            .transpose(1, 2, 4, 0, 3)         # p, s, c, r, n
            .reshape(4, 6, nrf * F)
        )
        xt[:, :, : nrf * F] = main.astype(ml_dtypes.bfloat16)
        if tp:
            tail = (
                xe[nrf * ROUND :]
                .reshape(tp, 2, F, 3)         # t, s, n, c
                .transpose(0, 1, 3, 2)        # t, s, c, n
                .reshape(tp, 6, F)
            )
            xt[:tp, :, nrf * F :] = tail.astype(ml_dtypes.bfloat16)

        # w1: 4 row strips (one per pair), each the [6,128] block-diag of W1'
        w1e = W1p[e].astype(ml_dtypes.bfloat16)
        w1_full = np.zeros((128, 128), dtype=ml_dtypes.bfloat16)
        for p in range(4):
            w1_full[32 * p : 32 * p + 3, 0:64] = w1e
            w1_full[32 * p + 3 : 32 * p + 6, 64:128] = w1e
        # w2: all four [64,64] quadrants hold W2 (L2 runs as 64x64-tile
        # matmuls reading whichever quadrant matches its tile position)
        w2_full = np.tile(W2[e].astype(ml_dtypes.bfloat16), (2, 2))
        # w3 normal/swapped: [128,4] each; normal maps partition half s ->
        # col s (even pairs), swapped maps half s -> col 1-s (odd pairs,
        # whose slots land swapped in psum after the quadrant L2)
        w3c = W3[e, :, 0].astype(ml_dtypes.bfloat16)
        w3e_full = np.zeros((128, 4), dtype=ml_dtypes.bfloat16)
        w3e_full[0:64, 0] = w3c
        w3e_full[64:128, 1] = w3c
        w3o_full = np.zeros((128, 4), dtype=ml_dtypes.bfloat16)
        w3o_full[64:128, 0] = w3c
        w3o_full[0:64, 1] = w3c
        wpack = np.ascontiguousarray(
            np.concatenate([w1_full, w2_full, w3e_full, w3o_full], axis=1)
        )
        b1_full = np.tile(b1p[e], 2).astype(np.float32)
        b2_full = np.tile(b2[e], 2).astype(np.float32)
        b3_full = np.full(128, b3[e, 0], dtype=np.float32)
        bpack = np.ascontiguousarray(
            np.stack([b1_full, b2_full, b3_full], axis=1)
        )
        in_maps.append({"xT": np.ascontiguousarray(xt), "wp": wpack,
                        "bp": bpack})

    shape_key = (nrf, tp)
    if shape_key not in _PROGRAM_CACHE:
        _PROGRAM_CACHE[shape_key] = _build_program(shape_key)
    nc = _PROGRAM_CACHE[shape_key]

    res = run_bass_kernel_spmd(nc, in_maps, core_ids=list(range(N_CORES)))
    LAST_RESULTS = res
    LAST_IN_MAPS = in_maps
    LAST_NC = nc
    LAST_SHAPE_KEY = shape_key

    # --- unshard: y[p, s, r*512+n] -> point r*4096 + (2p+s)*512 + n ---
    y_sorted = np.empty(n_pts, dtype=np.float32)
    for e in range(E):
        yd = res.results[e]["y"]              # [4, 2, W]
        flat = (
            yd.reshape(4, 2, n_sub, F)        # p, s, r, n
            .transpose(2, 0, 1, 3)            # r, p, s, n
            .reshape(n_sub * ROUND)
        )
        y_sorted[starts[e] : starts[e] + counts[e]] = flat[: counts[e]]

    y_full = np.empty(n_pts, dtype=np.float32)
    y_full[order] = y_sorted
    return y_full[:, None]


# revision 22
# speedup vs baseline: 1.2949x; 1.1493x over previous
"""Ensemble-SRN MoE routing kernel for 8 TRN2 NeuronCores.

Strategy: expert-parallel sharding. The 8 experts are axis-aligned octants of
[-1,1]^3 (GRID=(2,2,2)); core e receives exactly the points routed to expert e
(the all-to-all dispatch happens on the host as part of sharding), runs a dense
single-expert 3->64->64->1 ReLU MLP over its (padded) shard, and the host
inverse-permutes the outputs.

Device dataflow: the whole shard's x (bf16) and y (f32) stay resident in
SBUF, so HBM traffic is 4 input DMAs (prefetched on the Pool/SWDGE queue)
plus a few chunked output DMAs per pass -- each DMACopy costs a flat ~625ns
on the single HWDGE queue, which was the original bottleneck. In bench mode
the whole pass is ping-pong double-buffered across a 2x-unrolled hardware
loop so consecutive passes overlap.

Per "round" of 4096 points (4 pairs x 2 slots x 512), weights block-diag
doubled over the two slots so every 512-column PE stream covers 1024 points:
  L1 (K=6 ->M=128): 4 row-quadrant matmuls, pair p at tile_position (32p, 0),
                    into two [128,1024] psum tiles (pool bufs=2); the four
                    row-quadrant array tiles execute concurrently
  L2 (K=64 ->M=64): 2 concurrent 64x64-quadrant matmuls per pair; even pairs
                    use array tiles (0,0)/(64,64), odd pairs (0,64)/(64,0)
                    (odd pairs' slot halves land swapped in psum; a swapped
                    w3 restores slot order), two [128,1024] psum tiles
                    (pool bufs=2)
  L3 (K=128->M=2):  4 col-quadrant matmuls writing y for both slots into
                    bank C0 of the ph2a tile (already fully read by the h2
                    evac), emitted one round LATE so each engine's baked
                    instruction order interleaves two rounds
  PSUM evac split (the true pacing resource): DVE takes pair 0,1 columns of
  h1/h2, ACT pairs 2,3 plus the y bias-add (~2.7 vs ~3.0 us per round).
A final partial round of `tp` pairs (1024 points each) handles the remainder
beyond 4096*nrf, so the shard capacity is nrf*4096 + tp*1024.
"""

import ml_dtypes
import numpy as np

import concourse.bass as bass
import concourse.tile as tile
from concourse import bacc, mybir
from concourse.bass_utils import run_bass_kernel_spmd

F32 = mybir.dt.float32
BF16 = mybir.dt.bfloat16

N_CORES = 8
GRID = (2, 2, 2)
H = 64
F = 512                  # points per tile (one PSUM-bank free dim, fp32)
PAIR = 2 * F             # points per pair (two slots, block-diag)
ROUND = 4 * PAIR         # 4096 points per full round

_PROGRAM_CACHE = {}
LAST_RESULTS = None   # BassKernelResults of the last run (for test harness)
LAST_IN_MAPS = None   # per-core input dicts of the last run (for test harness)
LAST_NC = None        # compiled program of the last run (for test harness)
LAST_SHAPE_KEY = None # (nrf, tp) of the last run (for test harness)


def _build_program(shape_key, loop_n=None, stage="full"):
    """Build the SPMD program for shape_key=(nrf, tp): nrf full rounds plus a
    tail partial round of tp pairs. loop_n (bench only): repeat the whole body
    loop_n times in a hardware For_i so device time can be measured through
    the noisy axon dispatch path by differencing two loop counts."""
    nrf, tp = shape_key
    n_sub = nrf + (1 if tp else 0)
    W = n_sub * F
    nc = bacc.Bacc(
        "TRN2",
        target_bir_lowering=False,
        debug=False,
        num_devices=N_CORES,
    )
    xT = nc.dram_tensor("xT", [4, 6, W], BF16, kind="ExternalInput")
    wp = nc.dram_tensor("wp", [128, 264], BF16, kind="ExternalInput")
    bp = nc.dram_tensor("bp", [128, 3], F32, kind="ExternalInput")
    yO = nc.dram_tensor("y", [4, 2, W], F32, kind="ExternalOutput")

    RELU = mybir.ActivationFunctionType.Relu
    IDENT = mybir.ActivationFunctionType.Identity
    ADD = mybir.AluOpType.add
    MAX = mybir.AluOpType.max

    def pairs_in(r):
        return 4 if r < nrf else tp

    with tile.TileContext(nc) as tc:
        with (
            tc.tile_pool(name="const", bufs=1) as const,
            tc.tile_pool(name="xin", bufs=2) as xin,
            tc.tile_pool(name="yout", bufs=2) as yout,
            tc.tile_pool(name="h1p", bufs=3) as h1pool,
            tc.tile_pool(name="h2p", bufs=3) as h2pool,
            tc.tile_pool(name="psH1", bufs=2, space="PSUM") as psH1,
            tc.tile_pool(name="psH2", bufs=2, space="PSUM") as psH2,
        ):
            w_sb = const.tile([128, 264], BF16)
            nc.sync.dma_start(w_sb[:], wp.ap())
            b_sb = const.tile([128, 3], F32)
            nc.sync.dma_start(b_sb[:], bp.ap())

            w1s = w_sb[:, 0:128]
            w2s = w_sb[:, 128:256]
            w3e = w_sb[:, 256:260]
            w3o = w_sb[:, 260:264]
            b1s = b_sb[:, 0:1]
            b2s = b_sb[:, 1:2]
            b3s = b_sb[:, 2:3]

            # Load the Relu/Identity activation table once, outside the loop,
            # so the per-pass body doesn't pay the ~1.3us table load.
            warm_a = const.tile([128, 1], F32)
            warm_b = const.tile([128, 1], F32)
            nc.vector.memset(warm_a[:], 0.0)
            nc.scalar.activation(
                warm_b[:], warm_a[:], mybir.ActivationFunctionType.Relu
            )
            nc.scalar.activation(
                warm_a[:], warm_b[:], mybir.ActivationFunctionType.Identity
            )

            import contextlib
            if loop_n:
                assert loop_n % 2 == 0 and loop_n >= 2, loop_n
                loop_cm = tc.For_i(
                    0, loop_n // 2, 1,
                    hint_engines=(
                        mybir.EngineType.PE,
                        mybir.EngineType.DVE,
                        mybir.EngineType.Activation,
                        mybir.EngineType.SP,
                        mybir.EngineType.Pool,
                    ),
                )
            else:
                loop_cm = contextlib.nullcontext()

            def emit_pass():
                # Ping-pong whole-shard buffers: pass i+1's x prefetch (on the
                # idle Pool/SWDGE queue, so it doesn't serialize behind the y
                # stores on SP) overlaps pass i's compute.
                x_sb = xin.tile([128, W], BF16, name="x_sb")
                y_sb = yout.tile([128, W], F32, name="y_sb")
                for p in range(4):
                    nc.gpsimd.dma_start(
                        x_sb[32 * p : 32 * p + 6, :], xT.ap()[p]
                    )

                # L3(q)+y(q) are emitted one round LATE (during round q+1),
                # so each engine's baked instruction order interleaves two
                # rounds and no op waits on work emitted after it: ACT runs
                # y(q), h1b(q+1), h2b(q+1); PE runs L1(q+1), L3(q), L2(q+1).
                def emit_l3_y(prev, yc0):
                    h2r_p, ph2a_p, q, np_q = prev
                    for p in range(np_q):
                        nc.tensor.matmul(
                            ph2a_p[32 * p : 32 * p + 2, 0:512],
                            w3e[:, 0:2] if p % 2 == 0 else w3o[:, 0:2],
                            h2r_p[:, F * p : F * p + F],
                            start=True,
                            stop=True,
                            tile_position=(0, 32 * p),
                        )
                    nc.scalar.activation(
                        y_sb[:, F * q : F * q + F], ph2a_p[:, 0:512],
                        IDENT, bias=b3s,
                    )
                    # chunked y store: flush every 8 rounds
                    if q == n_sub - 1 or (q % 8) == 7:
                        c1 = F * q + F
                        for p in range(4):
                            nc.sync.dma_start(
                                yO.ap()[p, :, yc0:c1],
                                y_sb[32 * p : 32 * p + 2, yc0:c1],
                            )
                        return c1
                    return yc0

                yc0 = 0
                prev = None
                for r in range(n_sub):
                    np_r = pairs_in(r)
                    cols = slice(F * r, F * r + F)

                    # ---- L1: np_r row-quadrant block-diag matmuls ----
                    ph1a = psH1.tile([128, 1024], F32, tag="ph1")
                    ph1b = (
                        psH1.tile([128, 1024], F32, tag="ph1", name="ph1b")
                        if np_r > 2 else None
                    )
                    for p in range(np_r):
                        dst = ph1a if p < 2 else ph1b
                        nc.tensor.matmul(
                            dst[:, F * (p % 2) : F * (p % 2) + F],
                            w1s[32 * p : 32 * p + 6, :],
                            x_sb[32 * p : 32 * p + 6, cols],
                            start=True,
                            stop=True,
                            tile_position=(32 * p, 0),
                        )

                    # ---- delayed L3/y of the previous round ----
                    if prev is not None:
                        yc0 = emit_l3_y(prev, yc0)

                    # relu + bias evac: DVE takes pairs 0,1; ACT pairs 2,3
                    h1r = h1pool.tile([128, 2048], BF16)
                    d_end = min(2, np_r) * F
                    nc.vector.tensor_scalar(
                        h1r[:, 0:d_end], ph1a[:, 0:d_end], b1s, 0.0, ADD, MAX
                    )
                    if np_r > 2:
                        nc.scalar.activation(
                            h1r[:, 1024 : F * np_r],
                            ph1b[:, 0 : F * (np_r - 2)],
                            RELU, bias=b1s,
                        )

                    # ---- L2: 2 concurrent 64x64-quadrant matmuls per pair
                    # (even pairs use tiles (0,0)/(64,64); odd pairs use
                    # (0,64)/(64,0), landing their slots swapped in psum --
                    # the swapped w3 below puts y back in slot order) ----
                    h2r = h2pool.tile([128, 2048], BF16)
                    ph2a = psH2.tile([128, 1024], F32, tag="ph2")
                    ph2b = (
                        psH2.tile([128, 1024], F32, tag="ph2", name="ph2b")
                        if np_r > 2 else None
                    )
                    for p in range(np_r):
                        dst = ph2a if p < 2 else ph2b
                        c = slice(F * (p % 2), F * (p % 2) + F)
                        for s in range(2):
                            if p % 2 == 0:
                                out_half = dst[64 * s : 64 * s + 64, c]
                                wq = w2s[64 * s : 64 * s + 64,
                                         64 * s : 64 * s + 64]
                                tpos = (64 * s, 64 * s)
                            else:
                                out_half = dst[64 - 64 * s : 128 - 64 * s, c]
                                wq = w2s[64 * s : 64 * s + 64,
                                         64 - 64 * s : 128 - 64 * s]
                                tpos = (64 * s, 64 - 64 * s)
                            nc.tensor.matmul(
                                out_half,
                                wq,
                                h1r[64 * s : 64 * s + 64, F * p : F * p + F],
                                start=True,
                                stop=True,
                                tile_position=tpos,
                            )
                    nc.vector.tensor_scalar(
                        h2r[:, 0:d_end], ph2a[:, 0:d_end], b2s, 0.0, ADD, MAX
                    )
                    if np_r > 2:
                        nc.scalar.activation(
                            h2r[:, 1024 : F * np_r],
                            ph2b[:, 0 : F * (np_r - 2)],
                            RELU, bias=b2s,
                        )

                    # L3(r) writes y into bank C0 of ph2a (fully read by the
                    # h2 evac; its slot isn't rewritten until round r+1's L2)
                    # -- emitted at the top of round r+1.
                    prev = (h2r, ph2a, r, np_r)

                emit_l3_y(prev, yc0)

            with loop_cm:
                emit_pass()
                if loop_n:
                    emit_pass()

    nc.compile()
    return nc


def kernel(x, extents_min, extents_max, W1, b1, W2, b2, W3, b3):
    global LAST_RESULTS, LAST_IN_MAPS, LAST_NC, LAST_SHAPE_KEY
    x = np.ascontiguousarray(np.asarray(x, dtype=np.float32))
    extents_min = np.asarray(extents_min, dtype=np.float32)
    extents_max = np.asarray(extents_max, dtype=np.float32)
    W1 = np.asarray(W1, dtype=np.float32)
    b1 = np.asarray(b1, dtype=np.float32)
    W2 = np.asarray(W2, dtype=np.float32)
    b2 = np.asarray(b2, dtype=np.float32)
    W3 = np.asarray(W3, dtype=np.float32)
    b3 = np.asarray(b3, dtype=np.float32)

    n_pts = x.shape[0]
    E = W1.shape[0]
    assert E == N_CORES

    # --- routing (identical fp32 math to the reference) ---
    gvec = np.asarray(GRID, dtype=np.float32)
    u = np.clip((x + np.float32(1.0)) * np.float32(0.5), 0.0, 0.99)
    gi = (u * gvec).astype(np.int32)
    idx = gi[:, 0] + gi[:, 1] * GRID[0] + gi[:, 2] * (GRID[0] * GRID[1])

    order = np.argsort(idx, kind="stable")
    counts = np.bincount(idx, minlength=E)
    starts = np.concatenate([[0], np.cumsum(counts)[:-1]])
    x_sorted = x[order]

    total_pairs = max(1, int(np.ceil(counts.max() / PAIR)))
    nrf, tp = total_pairs // 4, total_pairs % 4
    if nrf == 0:
        nrf, tp = 1, 0
    cap = nrf * ROUND + tp * PAIR
    n_sub = nrf + (1 if tp else 0)
    W = n_sub * F

    # --- fold the expert-local normalization into layer-1 weights ---
    # xn = s*x + t, s = 2/(emax-emin), t = -2*emin/(emax-emin) - 1
    span = extents_max - extents_min          # [E, 3]
    s = 2.0 / span
    tvec = -2.0 * extents_min / span - 1.0
    # h1_pre = x @ W1e' + b1e',  W1e' = diag(s) @ W1e, b1e' = b1e + t @ W1e
    W1p = W1 * s[:, :, None]                  # [E, 3, H]
    b1p = b1 + np.einsum("ec,ech->eh", tvec, W1)

    in_maps = []
    for e in range(E):
        xe = np.zeros((cap, 3), dtype=np.float32)
        xe[: counts[e]] = x_sorted[starts[e] : starts[e] + counts[e]]
        # xt[p, 3s+c, r*512+n] = xe[r*4096 + (2p+s)*512 + n, c]
        xt = np.zeros((4, 6, W), dtype=ml_dtypes.bfloat16)
        main = (
            xe[: nrf * ROUND]
            .reshape(nrf, 4, 2, F, 3)         # r, p, s, n, c
            .transpose(1, 2, 4, 0, 3)         # p, s, c, r, n
            .reshape(4, 6, nrf * F)
        )
        xt[:, :, : nrf * F] = main.astype(ml_dtypes.bfloat16)
        if tp:
            tail = (
                xe[nrf * ROUND :]
                .reshape(tp, 2, F, 3)         # t, s, n, c
                .transpose(0, 1, 3, 2)        # t, s, c, n
                .reshape(tp, 6, F)
            )
            xt[:tp, :, nrf * F :] = tail.astype(ml_dtypes.bfloat16)

        # w1: 4 row strips (one per pair), each the [6,128] block-diag of W1'
        w1e = W1p[e].astype(ml_dtypes.bfloat16)
        w1_full = np.zeros((128, 128), dtype=ml_dtypes.bfloat16)
        for p in range(4):
            w1_full[32 * p : 32 * p + 3, 0:64] = w1e
            w1_full[32 * p + 3 : 32 * p + 6, 64:128] = w1e
        # w2: all four [64,64] quadrants hold W2 (L2 runs as 64x64-tile
        # matmuls reading whichever quadrant matches its tile position)
        w2_full = np.tile(W2[e].astype(ml_dtypes.bfloat16), (2, 2))
        # w3 normal/swapped: [128,4] each; normal maps partition half s ->
        # col s (even pairs), swapped maps half s -> col 1-s (odd pairs,
        # whose slots land swapped in psum after the quadrant L2)
        w3c = W3[e, :, 0].astype(ml_dtypes.bfloat16)
        w3e_full = np.zeros((128, 4), dtype=ml_dtypes.bfloat16)
        w3e_full[0:64, 0] = w3c
        w3e_full[64:128, 1] = w3c
        w3o_full = np.zeros((128, 4), dtype=ml_dtypes.bfloat16)
        w3o_full[64:128, 0] = w3c
        w3o_full[0:64, 1] = w3c
        wpack = np.ascontiguousarray(
            np.concatenate([w1_full, w2_full, w3e_full, w3o_full], axis=1)
        )
        b1_full = np.tile(b1p[e], 2).astype(np.float32)
        b2_full = np.tile(b2[e], 2).astype(np.float32)
        b3_full = np.full(128, b3[e, 0], dtype=np.float32)
        bpack = np.ascontiguousarray(
            np.stack([b1_full, b2_full, b3_full], axis=1)
        )
        in_maps.append({"xT": np.ascontiguousarray(xt), "wp": wpack,
                        "bp": bpack})

    shape_key = (nrf, tp)
    if shape_key not in _PROGRAM_CACHE:
        _PROGRAM_CACHE[shape_key] = _build_program(shape_key)
    nc = _PROGRAM_CACHE[shape_key]

    res = run_bass_kernel_spmd(nc, in_maps, core_ids=list(range(N_CORES)))
    LAST_RESULTS = res
    LAST_IN_MAPS = in_maps
    LAST_NC = nc
    LAST_SHAPE_KEY = shape_key

    # --- unshard: y[p, s, r*512+n] -> point r*4096 + (2p+s)*512 + n ---
    y_sorted = np.empty(n_pts, dtype=np.float32)
    for e in range(E):
        yd = res.results[e]["y"]              # [4, 2, W]
        flat = (
            yd.reshape(4, 2, n_sub, F)        # p, s, r, n
            .transpose(2, 0, 1, 3)            # r, p, s, n
            .reshape(n_sub * ROUND)
        )
        y_sorted[starts[e] : starts[e] + counts[e]] = flat[: counts[e]]

    y_full = np.empty(n_pts, dtype=np.float32)
    y_full[order] = y_sorted
    return y_full[:, None]


# revision 23
# speedup vs baseline: 1.3046x; 1.0075x over previous
"""Ensemble-SRN MoE routing kernel for 8 TRN2 NeuronCores.

Strategy: expert-parallel sharding. The 8 experts are axis-aligned octants of
[-1,1]^3 (GRID=(2,2,2)); core e receives exactly the points routed to expert e
(the all-to-all dispatch happens on the host as part of sharding), runs a dense
single-expert 3->64->64->1 ReLU MLP over its (padded) shard, and the host
inverse-permutes the outputs.

Device dataflow: the whole shard's x (bf16) and y (f32) stay resident in
SBUF, so HBM traffic is 4 input DMAs (prefetched on the Pool/SWDGE queue)
plus a few chunked output DMAs per pass -- each DMACopy costs a flat ~625ns
on the single HWDGE queue, which was the original bottleneck. In bench mode
the whole pass is ping-pong double-buffered across a 2x-unrolled hardware
loop so consecutive passes overlap.

Per "round" of 4096 points (4 pairs x 2 slots x 512), weights block-diag
doubled over the two slots so every 512-column PE stream covers 1024 points:
  L1 (K=6 ->M=128): 4 row-quadrant matmuls, pair p at tile_position (32p, 0),
                    into two [128,1024] psum tiles (pool bufs=2); the four
                    row-quadrant array tiles execute concurrently
  L2 (K=64 ->M=64): 2 concurrent 64x64-quadrant matmuls per pair; even pairs
                    use array tiles (0,0)/(64,64), odd pairs (0,64)/(64,0)
                    (odd pairs' slot halves land swapped in psum; a swapped
                    w3 restores slot order), two [128,1024] psum tiles
                    (pool bufs=2)
  L3 (K=128->M=2):  4 col-quadrant matmuls writing y for both slots into
                    bank C0 of the ph2a tile (already fully read by the h2
                    evac), emitted one round LATE so each engine's baked
                    instruction order interleaves two rounds
  PSUM evac split (the true pacing resource): DVE takes pair 0,1 columns of
  h1/h2, ACT pairs 2,3 plus the y bias-add (~2.7 vs ~3.0 us per round).
A final partial round of `tp` pairs (1024 points each) handles the remainder
beyond 4096*nrf, so the shard capacity is nrf*4096 + tp*1024.
"""

import ml_dtypes
import numpy as np

import concourse.bass as bass
import concourse.tile as tile
from concourse import bacc, mybir
from concourse.bass_utils import run_bass_kernel_spmd

F32 = mybir.dt.float32
BF16 = mybir.dt.bfloat16

N_CORES = 8
GRID = (2, 2, 2)
H = 64
F = 512                  # points per tile (one PSUM-bank free dim, fp32)
PAIR = 2 * F             # points per pair (two slots, block-diag)
ROUND = 4 * PAIR         # 4096 points per full round

_PROGRAM_CACHE = {}
LAST_RESULTS = None   # BassKernelResults of the last run (for test harness)
LAST_IN_MAPS = None   # per-core input dicts of the last run (for test harness)
LAST_NC = None        # compiled program of the last run (for test harness)
LAST_SHAPE_KEY = None # (nrf, tp) of the last run (for test harness)


def _build_program(shape_key, loop_n=None, stage="full"):
    """Build the SPMD program for shape_key=(nrf, tp): nrf full rounds plus a
    tail partial round of tp pairs. loop_n (bench only): repeat the whole body
    loop_n times in a hardware For_i so device time can be measured through
    the noisy axon dispatch path by differencing two loop counts."""
    nrf, tp = shape_key
    n_sub = nrf + (1 if tp else 0)
    W = n_sub * F
    nc = bacc.Bacc(
        "TRN2",
        target_bir_lowering=False,
        debug=False,
        num_devices=N_CORES,
    )
    xT = nc.dram_tensor("xT", [4, 6, W], BF16, kind="ExternalInput")
    wp = nc.dram_tensor("wp", [128, 264], BF16, kind="ExternalInput")
    bp = nc.dram_tensor("bp", [128, 3], F32, kind="ExternalInput")
    yO = nc.dram_tensor("y", [4, 2, W], F32, kind="ExternalOutput")

    RELU = mybir.ActivationFunctionType.Relu
    IDENT = mybir.ActivationFunctionType.Identity
    ADD = mybir.AluOpType.add
    MAX = mybir.AluOpType.max

    def pairs_in(r):
        return 4 if r < nrf else tp

    with tile.TileContext(nc) as tc:
        with (
            tc.tile_pool(name="const", bufs=1) as const,
            tc.tile_pool(name="xin", bufs=2) as xin,
            tc.tile_pool(name="yout", bufs=2) as yout,
            tc.tile_pool(name="h1p", bufs=3) as h1pool,
            tc.tile_pool(name="h2p", bufs=3) as h2pool,
            tc.tile_pool(name="psH1", bufs=2, space="PSUM") as psH1,
            tc.tile_pool(name="psH2", bufs=2, space="PSUM") as psH2,
        ):
            w_sb = const.tile([128, 264], BF16)
            nc.sync.dma_start(w_sb[:], wp.ap())
            b_sb = const.tile([128, 3], F32)
            nc.sync.dma_start(b_sb[:], bp.ap())

            w1s = w_sb[:, 0:128]
            w2s = w_sb[:, 128:256]
            w3e = w_sb[:, 256:260]
            w3o = w_sb[:, 260:264]
            b1s = b_sb[:, 0:1]
            b2s = b_sb[:, 1:2]
            b3s = b_sb[:, 2:3]

            # Load the Relu/Identity activation table once, outside the loop,
            # so the per-pass body doesn't pay the ~1.3us table load.
            warm_a = const.tile([128, 1], F32)
            warm_b = const.tile([128, 1], F32)
            nc.vector.memset(warm_a[:], 0.0)
            nc.scalar.activation(
                warm_b[:], warm_a[:], mybir.ActivationFunctionType.Relu
            )
            nc.scalar.activation(
                warm_a[:], warm_b[:], mybir.ActivationFunctionType.Identity
            )

            import contextlib
            if loop_n:
                assert loop_n % 2 == 0 and loop_n >= 2, loop_n
                loop_cm = tc.For_i(
                    0, loop_n // 2, 1,
                    hint_engines=(
                        mybir.EngineType.PE,
                        mybir.EngineType.DVE,
                        mybir.EngineType.Activation,
                        mybir.EngineType.SP,
                        mybir.EngineType.Pool,
                    ),
                )
            else:
                loop_cm = contextlib.nullcontext()

            def emit_pass():
                # Ping-pong whole-shard buffers: pass i+1's x prefetch (on the
                # idle Pool/SWDGE queue, so it doesn't serialize behind the y
                # stores on SP) overlaps pass i's compute.
                x_sb = xin.tile([128, W], BF16, name="x_sb")
                y_sb = yout.tile([128, W], F32, name="y_sb")
                for p in range(4):
                    nc.gpsimd.dma_start(
                        x_sb[32 * p : 32 * p + 6, :], xT.ap()[p]
                    )

                # L3(q)+y(q) are emitted one round LATE (during round q+1),
                # so each engine's baked instruction order interleaves two
                # rounds and no op waits on work emitted after it: ACT runs
                # y(q), h1b(q+1), h2b(q+1); PE runs L1(q+1), L3(q), L2(q+1).
                def emit_l3_y(prev, yc0):
                    h2r_p, ph2a_p, q, np_q = prev
                    for p in range(np_q):
                        nc.tensor.matmul(
                            ph2a_p[32 * p : 32 * p + 2, 0:512],
                            w3e[:, 0:2] if p % 2 == 0 else w3o[:, 0:2],
                            h2r_p[:, F * p : F * p + F],
                            start=True,
                            stop=True,
                            tile_position=(0, 32 * p),
                        )
                    nc.scalar.activation(
                        y_sb[:, F * q : F * q + F], ph2a_p[:, 0:512],
                        IDENT, bias=b3s,
                    )
                    # chunked y store: flush every 8 rounds
                    if q == n_sub - 1 or (q % 4) == 3:
                        c1 = F * q + F
                        for p in range(4):
                            nc.sync.dma_start(
                                yO.ap()[p, :, yc0:c1],
                                y_sb[32 * p : 32 * p + 2, yc0:c1],
                            )
                        return c1
                    return yc0

                yc0 = 0
                prev = None
                for r in range(n_sub):
                    np_r = pairs_in(r)
                    cols = slice(F * r, F * r + F)

                    # ---- L1: np_r row-quadrant block-diag matmuls ----
                    ph1a = psH1.tile([128, 1024], F32, tag="ph1")
                    ph1b = (
                        psH1.tile([128, 1024], F32, tag="ph1", name="ph1b")
                        if np_r > 2 else None
                    )
                    for p in range(np_r):
                        dst = ph1a if p < 2 else ph1b
                        nc.tensor.matmul(
                            dst[:, F * (p % 2) : F * (p % 2) + F],
                            w1s[32 * p : 32 * p + 6, :],
                            x_sb[32 * p : 32 * p + 6, cols],
                            start=True,
                            stop=True,
                            tile_position=(32 * p, 0),
                        )

                    # ---- delayed L3/y of the previous round ----
                    if prev is not None:
                        yc0 = emit_l3_y(prev, yc0)

                    # relu + bias evac: DVE takes pairs 0,1; ACT pairs 2,3
                    h1r = h1pool.tile([128, 2048], BF16)
                    d_end = min(2, np_r) * F
                    nc.vector.tensor_scalar(
                        h1r[:, 0:d_end], ph1a[:, 0:d_end], b1s, 0.0, ADD, MAX
                    )
                    if np_r > 2:
                        nc.scalar.activation(
                            h1r[:, 1024 : F * np_r],
                            ph1b[:, 0 : F * (np_r - 2)],
                            RELU, bias=b1s,
                        )

                    # ---- L2: 2 concurrent 64x64-quadrant matmuls per pair
                    # (even pairs use tiles (0,0)/(64,64); odd pairs use
                    # (0,64)/(64,0), landing their slots swapped in psum --
                    # the swapped w3 below puts y back in slot order) ----
                    h2r = h2pool.tile([128, 2048], BF16)
                    ph2a = psH2.tile([128, 1024], F32, tag="ph2")
                    ph2b = (
                        psH2.tile([128, 1024], F32, tag="ph2", name="ph2b")
                        if np_r > 2 else None
                    )
                    for p in range(np_r):
                        dst = ph2a if p < 2 else ph2b
                        c = slice(F * (p % 2), F * (p % 2) + F)
                        for s in range(2):
                            if p % 2 == 0:
                                out_half = dst[64 * s : 64 * s + 64, c]
                                wq = w2s[64 * s : 64 * s + 64,
                                         64 * s : 64 * s + 64]
                                tpos = (64 * s, 64 * s)
                            else:
                                out_half = dst[64 - 64 * s : 128 - 64 * s, c]
                                wq = w2s[64 * s : 64 * s + 64,
                                         64 - 64 * s : 128 - 64 * s]
                                tpos = (64 * s, 64 - 64 * s)
                            nc.tensor.matmul(
                                out_half,
                                wq,
                                h1r[64 * s : 64 * s + 64, F * p : F * p + F],
                                start=True,
                                stop=True,
                                tile_position=tpos,
                            )
                    nc.vector.tensor_scalar(
                        h2r[:, 0:d_end], ph2a[:, 0:d_end], b2s, 0.0, ADD, MAX
                    )
                    if np_r > 2:
                        nc.scalar.activation(
                            h2r[:, 1024 : F * np_r],
                            ph2b[:, 0 : F * (np_r - 2)],
                            RELU, bias=b2s,
                        )

                    # L3(r) writes y into bank C0 of ph2a (fully read by the
                    # h2 evac; its slot isn't rewritten until round r+1's L2)
                    # -- emitted at the top of round r+1.
                    prev = (h2r, ph2a, r, np_r)

                emit_l3_y(prev, yc0)

            with loop_cm:
                emit_pass()
                if loop_n:
                    emit_pass()

    nc.compile()
    return nc


def kernel(x, extents_min, extents_max, W1, b1, W2, b2, W3, b3):
    global LAST_RESULTS, LAST_IN_MAPS, LAST_NC, LAST_SHAPE_KEY
    x = np.ascontiguousarray(np.asarray(x, dtype=np.float32))
    extents_min = np.asarray(extents_min, dtype=np.float32)
    extents_max = np.asarray(extents_max, dtype=np.float32)
    W1 = np.asarray(W1, dtype=np.float32)
    b1 = np.asarray(b1, dtype=np.float32)
    W2 = np.asarray(W2, dtype=np.float32)
    b2 = np.asarray(b2, dtype=np.float32)
    W3 = np.asarray(W3, dtype=np.float32)
    b3 = np.asarray(b3, dtype=np.float32)

    n_pts = x.shape[0]
    E = W1.shape[0]
    assert E == N_CORES

    # --- routing (identical fp32 math to the reference) ---
    gvec = np.asarray(GRID, dtype=np.float32)
    u = np.clip((x + np.float32(1.0)) * np.float32(0.5), 0.0, 0.99)
    gi = (u * gvec).astype(np.int32)
    idx = gi[:, 0] + gi[:, 1] * GRID[0] + gi[:, 2] * (GRID[0] * GRID[1])

    order = np.argsort(idx, kind="stable")
    counts = np.bincount(idx, minlength=E)
    starts = np.concatenate([[0], np.cumsum(counts)[:-1]])
    x_sorted = x[order]

    total_pairs = max(1, int(np.ceil(counts.max() / PAIR)))
    nrf, tp = total_pairs // 4, total_pairs % 4
    if nrf == 0:
        nrf, tp = 1, 0
    cap = nrf * ROUND + tp * PAIR
    n_sub = nrf + (1 if tp else 0)
    W = n_sub * F

    # --- fold the expert-local normalization into layer-1 weights ---
    # xn = s*x + t, s = 2/(emax-emin), t = -2*emin/(emax-emin) - 1
    span = extents_max - extents_min          # [E, 3]
    s = 2.0 / span
    tvec = -2.0 * extents_min / span - 1.0
    # h1_pre = x @ W1e' + b1e',  W1e' = diag(s) @ W1e, b1e' = b1e + t @ W1e
    W1p = W1 * s[:, :, None]                  # [E, 3, H]
    b1p = b1 + np.einsum("ec,ech->eh", tvec, W1)

    in_maps = []
    for e in range(E):
        xe = np.zeros((cap, 3), dtype=np.float32)
        xe[: counts[e]] = x_sorted[starts[e] : starts[e] + counts[e]]
        # xt[p, 3s+c, r*512+n] = xe[r*4096 + (2p+s)*512 + n, c]
        xt = np.zeros((4, 6, W), dtype=ml_dtypes.bfloat16)
        main = (
            xe[: nrf * ROUND]
            .reshape(nrf, 4, 2, F, 3)         # r, p, s, n, c
            .transpose(1, 2, 4, 0, 3)         # p, s, c, r, n
            .reshape(4, 6, nrf * F)
        )
        xt[:, :, : nrf * F] = main.astype(ml_dtypes.bfloat16)
        if tp:
            tail = (
                xe[nrf * ROUND :]
                .reshape(tp, 2, F, 3)         # t, s, n, c
                .transpose(0, 1, 3, 2)        # t, s, c, n
                .reshape(tp, 6, F)
            )
            xt[:tp, :, nrf * F :] = tail.astype(ml_dtypes.bfloat16)

        # w1: 4 row strips (one per pair), each the [6,128] block-diag of W1'
        w1e = W1p[e].astype(ml_dtypes.bfloat16)
        w1_full = np.zeros((128, 128), dtype=ml_dtypes.bfloat16)
        for p in range(4):
            w1_full[32 * p : 32 * p + 3, 0:64] = w1e
            w1_full[32 * p + 3 : 32 * p + 6, 64:128] = w1e
        # w2: all four [64,64] quadrants hold W2 (L2 runs as 64x64-tile
        # matmuls reading whichever quadrant matches its tile position)
        w2_full = np.tile(W2[e].astype(ml_dtypes.bfloat16), (2, 2))
        # w3 normal/swapped: [128,4] each; normal maps partition half s ->
        # col s (even pairs), swapped maps half s -> col 1-s (odd pairs,
        # whose slots land swapped in psum after the quadrant L2)
        w3c = W3[e, :, 0].astype(ml_dtypes.bfloat16)
        w3e_full = np.zeros((128, 4), dtype=ml_dtypes.bfloat16)
        w3e_full[0:64, 0] = w3c
        w3e_full[64:128, 1] = w3c
        w3o_full = np.zeros((128, 4), dtype=ml_dtypes.bfloat16)
        w3o_full[64:128, 0] = w3c
        w3o_full[0:64, 1] = w3c
        wpack = np.ascontiguousarray(
            np.concatenate([w1_full, w2_full, w3e_full, w3o_full], axis=1)
        )
        b1_full = np.tile(b1p[e], 2).astype(np.float32)
        b2_full = np.tile(b2[e], 2).astype(np.float32)
        b3_full = np.full(128, b3[e, 0], dtype=np.float32)
        bpack = np.ascontiguousarray(
            np.stack([b1_full, b2_full, b3_full], axis=1)
        )
        in_maps.append({"xT": np.ascontiguousarray(xt), "wp": wpack,
                        "bp": bpack})

    shape_key = (nrf, tp)
    if shape_key not in _PROGRAM_CACHE:
        _PROGRAM_CACHE[shape_key] = _build_program(shape_key)
    nc = _PROGRAM_CACHE[shape_key]

    res = run_bass_kernel_spmd(nc, in_maps, core_ids=list(range(N_CORES)))
    LAST_RESULTS = res
    LAST_IN_MAPS = in_maps
    LAST_NC = nc
    LAST_SHAPE_KEY = shape_key

    # --- unshard: y[p, s, r*512+n] -> point r*4096 + (2p+s)*512 + n ---
    y_sorted = np.empty(n_pts, dtype=np.float32)
    for e in range(E):
        yd = res.results[e]["y"]              # [4, 2, W]
        flat = (
            yd.reshape(4, 2, n_sub, F)        # p, s, r, n
            .transpose(2, 0, 1, 3)            # r, p, s, n
            .reshape(n_sub * ROUND)
        )
        y_sorted[starts[e] : starts[e] + counts[e]] = flat[: counts[e]]

    y_full = np.empty(n_pts, dtype=np.float32)
    y_full[order] = y_sorted
    return y_full[:, None]


# revision 24
# speedup vs baseline: 1.3992x; 1.0725x over previous
"""Ensemble-SRN MoE routing kernel for 8 TRN2 NeuronCores.

Strategy: expert-parallel sharding. The 8 experts are axis-aligned octants of
[-1,1]^3 (GRID=(2,2,2)); core e receives exactly the points routed to expert e
(the all-to-all dispatch happens on the host as part of sharding), runs a dense
single-expert 3->64->64->1 ReLU MLP over its (padded) shard, and the host
inverse-permutes the outputs.

Device dataflow: the whole shard's x (bf16) and y (f32) stay resident in
SBUF, so HBM traffic is 4 input DMAs (prefetched on the Pool/SWDGE queue)
plus a few chunked output DMAs per pass -- each DMACopy costs a flat ~625ns
on the single HWDGE queue, which was the original bottleneck. In bench mode
the whole pass is ping-pong double-buffered across a 2x-unrolled hardware
loop so consecutive passes overlap.

Per "round" of 4096 points (4 pairs x 2 slots x 512), weights block-diag
doubled over the two slots so every 512-column PE stream covers 1024 points:
  L1 (K=6 ->M=128): 4 row-quadrant matmuls, pair p at tile_position (32p, 0),
                    into two [128,1024] psum tiles (pool bufs=2); the four
                    row-quadrant array tiles execute concurrently
  L2 (K=64 ->M=64): 2 concurrent 64x64-quadrant matmuls per pair; even pairs
                    use array tiles (0,0)/(64,64), odd pairs (0,64)/(64,0)
                    (odd pairs' slot halves land swapped in psum; a swapped
                    w3 restores slot order), two [128,1024] psum tiles
                    (pool bufs=2)
  L3 (K=128->M=2):  4 col-quadrant matmuls writing y for both slots into
                    bank C0 of the ph2a tile (already fully read by the h2
                    evac), emitted one round LATE so each engine's baked
                    instruction order interleaves two rounds
  PSUM evac split (the true pacing resource): DVE takes pair 0,1 columns of
  h1/h2, ACT pairs 2,3 plus the y bias-add (~2.7 vs ~3.0 us per round).
A final partial round of `tp` pairs (1024 points each) handles the remainder
beyond 4096*nrf, so the shard capacity is nrf*4096 + tp*1024.
"""

import ml_dtypes
import numpy as np

import concourse.bass as bass
import concourse.tile as tile
from concourse import bacc, mybir
from concourse.bass_utils import run_bass_kernel_spmd

F32 = mybir.dt.float32
BF16 = mybir.dt.bfloat16

N_CORES = 8
GRID = (2, 2, 2)
H = 64
F = 512                  # points per tile (one PSUM-bank free dim, fp32)
PAIR = 2 * F             # points per pair (two slots, block-diag)
ROUND = 4 * PAIR         # 4096 points per full round

_PROGRAM_CACHE = {}
LAST_RESULTS = None   # BassKernelResults of the last run (for test harness)
LAST_IN_MAPS = None   # per-core input dicts of the last run (for test harness)
LAST_NC = None        # compiled program of the last run (for test harness)
LAST_SHAPE_KEY = None # (nrf, tp) of the last run (for test harness)


def _build_program(shape_key, loop_n=None, stage="full"):
    """Build the SPMD program for shape_key=(nrf, tp): nrf full rounds plus a
    tail partial round of tp pairs. loop_n (bench only): repeat the whole body
    loop_n times in a hardware For_i so device time can be measured through
    the noisy axon dispatch path by differencing two loop counts."""
    nrf, tp = shape_key
    n_sub = nrf + (1 if tp else 0)
    W = n_sub * F
    nc = bacc.Bacc(
        "TRN2",
        target_bir_lowering=False,
        debug=False,
        num_devices=N_CORES,
    )
    xT = nc.dram_tensor("xT", [4, 6, W], BF16, kind="ExternalInput")
    wp = nc.dram_tensor("wp", [128, 264], BF16, kind="ExternalInput")
    bp = nc.dram_tensor("bp", [128, 3], F32, kind="ExternalInput")
    yO = nc.dram_tensor("y", [4, 2, W], F32, kind="ExternalOutput")

    RELU = mybir.ActivationFunctionType.Relu
    IDENT = mybir.ActivationFunctionType.Identity
    ADD = mybir.AluOpType.add
    MAX = mybir.AluOpType.max

    def pairs_in(r):
        return 4 if r < nrf else tp

    with tile.TileContext(nc) as tc:
        with (
            tc.tile_pool(name="const", bufs=1) as const,
            tc.tile_pool(name="xin", bufs=2) as xin,
            tc.tile_pool(name="yout", bufs=2) as yout,
            tc.tile_pool(name="h1p", bufs=3) as h1pool,
            tc.tile_pool(name="h2p", bufs=3) as h2pool,
            tc.tile_pool(name="psH1", bufs=2, space="PSUM") as psH1,
            tc.tile_pool(name="psH2", bufs=2, space="PSUM") as psH2,
        ):
            w_sb = const.tile([128, 264], BF16)
            nc.sync.dma_start(w_sb[:], wp.ap())
            b_sb = const.tile([128, 3], F32)
            nc.sync.dma_start(b_sb[:], bp.ap())

            w1s = w_sb[:, 0:128]
            w2s = w_sb[:, 128:256]
            w3e = w_sb[:, 256:260]
            w3o = w_sb[:, 260:264]
            b1s = b_sb[:, 0:1]
            b2s = b_sb[:, 1:2]
            b3s = b_sb[:, 2:3]

            # Load the Relu/Identity activation table once, outside the loop,
            # so the per-pass body doesn't pay the ~1.3us table load.
            warm_a = const.tile([128, 1], F32)
            warm_b = const.tile([128, 1], F32)
            nc.vector.memset(warm_a[:], 0.0)
            nc.scalar.activation(
                warm_b[:], warm_a[:], mybir.ActivationFunctionType.Relu
            )
            nc.scalar.activation(
                warm_a[:], warm_b[:], mybir.ActivationFunctionType.Identity
            )

            import contextlib
            if loop_n:
                assert loop_n % 2 == 0 and loop_n >= 2, loop_n
                loop_cm = tc.For_i(
                    0, loop_n // 2, 1,
                    hint_engines=(
                        mybir.EngineType.PE,
                        mybir.EngineType.DVE,
                        mybir.EngineType.Activation,
                        mybir.EngineType.SP,
                        mybir.EngineType.Pool,
                    ),
                )
            else:
                loop_cm = contextlib.nullcontext()

            def emit_pass():
                # Ping-pong whole-shard buffers: pass i+1's x prefetch (on the
                # idle Pool/SWDGE queue, so it doesn't serialize behind the y
                # stores on SP) overlaps pass i's compute.
                x_sb = xin.tile([128, W], BF16, name="x_sb")
                y_sb = yout.tile([128, W], F32, name="y_sb")
                for p in range(4):
                    nc.gpsimd.dma_start(
                        x_sb[32 * p : 32 * p + 6, :], xT.ap()[p]
                    )

                # L3(q)+y(q) are emitted one round LATE (during round q+1),
                # so each engine's baked instruction order interleaves two
                # rounds and no op waits on work emitted after it: ACT runs
                # y(q), h1b(q+1), h2b(q+1); PE runs L1(q+1), L3(q), L2(q+1).
                def emit_l3_y(prev, yc0):
                    h2r_p, ph2a_p, ph2b_p, q, np_q = prev
                    # y lives in bank D0 (ph2b) when it exists: its slot's
                    # next writer is round q+1's SECOND L2 wave, giving the
                    # y bias-add more slack than C0 (rewritten by the first).
                    phy = ph2b_p if ph2b_p is not None else ph2a_p
                    for p in range(np_q):
                        nc.tensor.matmul(
                            phy[32 * p : 32 * p + 2, 0:512],
                            w3e[:, 0:2] if p % 2 == 0 else w3o[:, 0:2],
                            h2r_p[:, F * p : F * p + F],
                            start=True,
                            stop=True,
                            tile_position=(0, 32 * p),
                        )
                    nc.scalar.activation(
                        y_sb[:, F * q : F * q + F], phy[:, 0:512],
                        IDENT, bias=b3s,
                    )
                    # chunked y store: flush every 8 rounds
                    if q == n_sub - 1 or (q % 4) == 3:
                        c1 = F * q + F
                        for p in range(4):
                            nc.sync.dma_start(
                                yO.ap()[p, :, yc0:c1],
                                y_sb[32 * p : 32 * p + 2, yc0:c1],
                            )
                        return c1
                    return yc0

                yc0 = 0
                prev = None
                for r in range(n_sub):
                    np_r = pairs_in(r)
                    cols = slice(F * r, F * r + F)

                    # ---- L1: np_r row-quadrant block-diag matmuls ----
                    ph1a = psH1.tile([128, 1024], F32, tag="ph1")
                    ph1b = (
                        psH1.tile([128, 1024], F32, tag="ph1", name="ph1b")
                        if np_r > 2 else None
                    )
                    for p in range(np_r):
                        dst = ph1a if p < 2 else ph1b
                        nc.tensor.matmul(
                            dst[:, F * (p % 2) : F * (p % 2) + F],
                            w1s[32 * p : 32 * p + 6, :],
                            x_sb[32 * p : 32 * p + 6, cols],
                            start=True,
                            stop=True,
                            tile_position=(32 * p, 0),
                        )

                    # ---- delayed L3/y of the previous round ----
                    if prev is not None:
                        yc0 = emit_l3_y(prev, yc0)

                    # relu + bias evac: DVE takes pairs 0,1; ACT pairs 2,3
                    h1r = h1pool.tile([128, 2048], BF16)
                    d_end = min(2, np_r) * F
                    nc.vector.tensor_scalar(
                        h1r[:, 0:d_end], ph1a[:, 0:d_end], b1s, 0.0, ADD, MAX
                    )
                    if np_r > 2:
                        nc.scalar.activation(
                            h1r[:, 1024 : F * np_r],
                            ph1b[:, 0 : F * (np_r - 2)],
                            RELU, bias=b1s,
                        )

                    # ---- L2: 2 concurrent 64x64-quadrant matmuls per pair
                    # (even pairs use tiles (0,0)/(64,64); odd pairs use
                    # (0,64)/(64,0), landing their slots swapped in psum --
                    # the swapped w3 below puts y back in slot order) ----
                    h2r = h2pool.tile([128, 2048], BF16)
                    ph2a = psH2.tile([128, 1024], F32, tag="ph2")
                    ph2b = (
                        psH2.tile([128, 1024], F32, tag="ph2", name="ph2b")
                        if np_r > 2 else None
                    )
                    for p in range(np_r):
                        dst = ph2a if p < 2 else ph2b
                        c = slice(F * (p % 2), F * (p % 2) + F)
                        for s in range(2):
                            if p % 2 == 0:
                                out_half = dst[64 * s : 64 * s + 64, c]
                                wq = w2s[64 * s : 64 * s + 64,
                                         64 * s : 64 * s + 64]
                                tpos = (64 * s, 64 * s)
                            else:
                                out_half = dst[64 - 64 * s : 128 - 64 * s, c]
                                wq = w2s[64 * s : 64 * s + 64,
                                         64 - 64 * s : 128 - 64 * s]
                                tpos = (64 * s, 64 - 64 * s)
                            nc.tensor.matmul(
                                out_half,
                                wq,
                                h1r[64 * s : 64 * s + 64, F * p : F * p + F],
                                start=True,
                                stop=True,
                                tile_position=tpos,
                            )
                    nc.vector.tensor_scalar(
                        h2r[:, 0:d_end], ph2a[:, 0:d_end], b2s, 0.0, ADD, MAX
                    )
                    if np_r > 2:
                        nc.scalar.activation(
                            h2r[:, 1024 : F * np_r],
                            ph2b[:, 0 : F * (np_r - 2)],
                            RELU, bias=b2s,
                        )

                    # L3(r) writes y into bank C0 of ph2a (fully read by the
                    # h2 evac; its slot isn't rewritten until round r+1's L2)
                    # -- emitted at the top of round r+1.
                    prev = (h2r, ph2a, ph2b, r, np_r)

                emit_l3_y(prev, yc0)

            with loop_cm:
                emit_pass()
                if loop_n:
                    emit_pass()

    nc.compile()
    return nc


def kernel(x, extents_min, extents_max, W1, b1, W2, b2, W3, b3):
    global LAST_RESULTS, LAST_IN_MAPS, LAST_NC, LAST_SHAPE_KEY
    x = np.ascontiguousarray(np.asarray(x, dtype=np.float32))
    extents_min = np.asarray(extents_min, dtype=np.float32)
    extents_max = np.asarray(extents_max, dtype=np.float32)
    W1 = np.asarray(W1, dtype=np.float32)
    b1 = np.asarray(b1, dtype=np.float32)
    W2 = np.asarray(W2, dtype=np.float32)
    b2 = np.asarray(b2, dtype=np.float32)
    W3 = np.asarray(W3, dtype=np.float32)
    b3 = np.asarray(b3, dtype=np.float32)

    n_pts = x.shape[0]
    E = W1.shape[0]
    assert E == N_CORES

    # --- routing (identical fp32 math to the reference) ---
    gvec = np.asarray(GRID, dtype=np.float32)
    u = np.clip((x + np.float32(1.0)) * np.float32(0.5), 0.0, 0.99)
    gi = (u * gvec).astype(np.int32)
    idx = gi[:, 0] + gi[:, 1] * GRID[0] + gi[:, 2] * (GRID[0] * GRID[1])

    order = np.argsort(idx, kind="stable")
    counts = np.bincount(idx, minlength=E)
    starts = np.concatenate([[0], np.cumsum(counts)[:-1]])
    x_sorted = x[order]

    total_pairs = max(1, int(np.ceil(counts.max() / PAIR)))
    nrf, tp = total_pairs // 4, total_pairs % 4
    if nrf == 0:
        nrf, tp = 1, 0
    cap = nrf * ROUND + tp * PAIR
    n_sub = nrf + (1 if tp else 0)
    W = n_sub * F

    # --- fold the expert-local normalization into layer-1 weights ---
    # xn = s*x + t, s = 2/(emax-emin), t = -2*emin/(emax-emin) - 1
    span = extents_max - extents_min          # [E, 3]
    s = 2.0 / span
    tvec = -2.0 * extents_min / span - 1.0
    # h1_pre = x @ W1e' + b1e',  W1e' = diag(s) @ W1e, b1e' = b1e + t @ W1e
    W1p = W1 * s[:, :, None]                  # [E, 3, H]
    b1p = b1 + np.einsum("ec,ech->eh", tvec, W1)

    in_maps = []
    for e in range(E):
        xe = np.zeros((cap, 3), dtype=np.float32)
        xe[: counts[e]] = x_sorted[starts[e] : starts[e] + counts[e]]
        # xt[p, 3s+c, r*512+n] = xe[r*4096 + (2p+s)*512 + n, c]
        xt = np.zeros((4, 6, W), dtype=ml_dtypes.bfloat16)
        main = (
            xe[: nrf * ROUND]
            .reshape(nrf, 4, 2, F, 3)         # r, p, s, n, c
            .transpose(1, 2, 4, 0, 3)         # p, s, c, r, n
            .reshape(4, 6, nrf * F)
        )
        xt[:, :, : nrf * F] = main.astype(ml_dtypes.bfloat16)
        if tp:
            tail = (
                xe[nrf * ROUND :]
                .reshape(tp, 2, F, 3)         # t, s, n, c
                .transpose(0, 1, 3, 2)        # t, s, c, n
                .reshape(tp, 6, F)
            )
            xt[:tp, :, nrf * F :] = tail.astype(ml_dtypes.bfloat16)

        # w1: 4 row strips (one per pair), each the [6,128] block-diag of W1'
        w1e = W1p[e].astype(ml_dtypes.bfloat16)
        w1_full = np.zeros((128, 128), dtype=ml_dtypes.bfloat16)
        for p in range(4):
            w1_full[32 * p : 32 * p + 3, 0:64] = w1e
            w1_full[32 * p + 3 : 32 * p + 6, 64:128] = w1e
        # w2: all four [64,64] quadrants hold W2 (L2 runs as 64x64-tile
        # matmuls reading whichever quadrant matches its tile position)
        w2_full = np.tile(W2[e].astype(ml_dtypes.bfloat16), (2, 2))
        # w3 normal/swapped: [128,4] each; normal maps partition half s ->
        # col s (even pairs), swapped maps half s -> col 1-s (odd pairs,
        # whose slots land swapped in psum after the quadrant L2)
        w3c = W3[e, :, 0].astype(ml_dtypes.bfloat16)
        w3e_full = np.zeros((128, 4), dtype=ml_dtypes.bfloat16)
        w3e_full[0:64, 0] = w3c
        w3e_full[64:128, 1] = w3c
        w3o_full = np.zeros((128, 4), dtype=ml_dtypes.bfloat16)
        w3o_full[64:128, 0] = w3c
        w3o_full[0:64, 1] = w3c
        wpack = np.ascontiguousarray(
            np.concatenate([w1_full, w2_full, w3e_full, w3o_full], axis=1)
        )
        b1_full = np.tile(b1p[e], 2).astype(np.float32)
        b2_full = np.tile(b2[e], 2).astype(np.float32)
        b3_full = np.full(128, b3[e, 0], dtype=np.float32)
        bpack = np.ascontiguousarray(
            np.stack([b1_full, b2_full, b3_full], axis=1)
        )
        in_maps.append({"xT": np.ascontiguousarray(xt), "wp": wpack,
                        "bp": bpack})

    shape_key = (nrf, tp)
    if shape_key not in _PROGRAM_CACHE:
        _PROGRAM_CACHE[shape_key] = _build_program(shape_key)
    nc = _PROGRAM_CACHE[shape_key]

    res = run_bass_kernel_spmd(nc, in_maps, core_ids=list(range(N_CORES)))
    LAST_RESULTS = res
    LAST_IN_MAPS = in_maps
    LAST_NC = nc
    LAST_SHAPE_KEY = shape_key

    # --- unshard: y[p, s, r*512+n] -> point r*4096 + (2p+s)*512 + n ---
    y_sorted = np.empty(n_pts, dtype=np.float32)
    for e in range(E):
        yd = res.results[e]["y"]              # [4, 2, W]
        flat = (
            yd.reshape(4, 2, n_sub, F)        # p, s, r, n
            .transpose(2, 0, 1, 3)            # r, p, s, n
            .reshape(n_sub * ROUND)
        )
        y_sorted[starts[e] : starts[e] + counts[e]] = flat[: counts[e]]

    y_full = np.empty(n_pts, dtype=np.float32)
    y_full[order] = y_sorted
    return y_full[:, None]
